# revision 8
# baseline (speedup 1.0000x reference)
"""Trainium2 Bass kernel for EntityMarker segment-reduce (span means).

Problem: sequence_output [128, 2048, 768] f32, entity_positions [128, 4] int.
For each batch b, compute the mean of sequence_output[b, s:e+1, :] for the
head span (cols 0,1) and tail span (cols 2,3), clamped like the reference.
Output: (head [128, 768], tail [128, 768]) f32.

Strategy (data-parallel over batch, 8 cores, load-balanced):
  - On host, compute clamped spans; per batch the union of the two spans is
    1-2 contiguous runs of rows. Only those rows (~26% of the tensor) are
    read on device. Batches are assigned to cores by greedy LPT on union
    size (16 batches/core) to balance per-core bytes.
  - Each run is covered by K-row windows. A gpsimd indirect DMA
    (InstDMACopy + dynamic offset) gathers one window per partition:
    out[p, :] = x[start[p] : start[p]+K] - 128 windows / instruction,
    48KB-contiguous HBM reads per descriptor.
  - Per window column j, a PE matmul accumulates W_j[128, 32]^T @ rows into
    PSUM [32, 768]: 32 segment means (16 head + 16 tail) per core. Weights
    are 1/span_len for rows inside the span and 0 for overhang/padding, so
    PSUM holds the means directly.
  - The device program is UNIFORM across cores (SPMD); all data-dependence
    is carried via input tensors (x shard, window starts, weights).
"""

import numpy as np

import os

_B, _L, _H = 128, 2048, 768
_NCORES = 8
_BPC = _B // _NCORES  # batches per core
_SEG = 2 * _BPC       # segments per core: 16 head + 16 tail
_K = int(os.environ.get("KERNEL_K", "16"))  # rows per window
_GBUFS = int(os.environ.get("KERNEL_GBUFS", "3"))

_prog_cache = {}


def _build_program(n_win):
    import concourse.bass as bass
    import concourse.mybir as mybir
    from concourse import bacc, tile

    f32 = mybir.dt.float32
    i32 = mybir.dt.int32
    n_inst = (n_win + 127) // 128
    p_last = n_win - (n_inst - 1) * 128
    n_mm = n_inst * _K  # matmul chunk slots (last instruction partial)

    nc = bacc.Bacc(None, target_bir_lowering=False)
    x = nc.declare_dram_parameter("x", [_BPC * _L, _H], f32, isOutput=False)
    idx = nc.declare_dram_parameter("idx", [128, n_inst], i32, isOutput=False)
    w = nc.declare_dram_parameter("w", [128, n_mm * _SEG], f32, isOutput=False)
    out = nc.declare_dram_parameter("out", [_SEG, _H], f32, isOutput=True)

    with tile.TileContext(nc) as tc:
        with (
            tc.tile_pool(name="const", bufs=1) as cpool,
            tc.tile_pool(name="gather", bufs=_GBUFS) as gpool,
            tc.tile_pool(name="psum", bufs=1, space="PSUM") as ppool,
        ):
            idx_t = cpool.tile([128, n_inst], i32)
            nc.sync.dma_start(out=idx_t[:], in_=idx[:])
            w_t = cpool.tile([128, n_mm * _SEG], f32)
            nc.sync.dma_start(out=w_t[:], in_=w[:])

            ps_a = ppool.tile([_SEG, 512], f32)
            ps_b = ppool.tile([_SEG, 256], f32)

            for t in range(n_inst):
                p = 128 if t < n_inst - 1 else p_last
                # NOTE: the gather out AP must be 2D — a 3D [128, K, H]
                # AP mis-gathers on HW (sim doesn't model it).
                g = gpool.tile([128, _K * _H], f32, tag="g")
                nc.gpsimd.indirect_dma_start(
                    out=g[:p],
                    out_offset=None,
                    in_=x[:],
                    in_offset=bass.IndirectOffsetOnAxis(
                        ap=idx_t[:p, t:t + 1], axis=0),
                )
                for j in range(_K):
                    c = t * _K + j
                    lhsT = w_t[:p, c * _SEG:(c + 1) * _SEG]
                    nc.tensor.matmul(
                        ps_a[:], lhsT, g[:p, j * _H:j * _H + 512],
                        start=(c == 0), stop=(c == n_mm - 1),
                    )
                    nc.tensor.matmul(
                        ps_b[:], lhsT, g[:p, j * _H + 512:(j + 1) * _H],
                        start=(c == 0), stop=(c == n_mm - 1),
                    )

            o_t = cpool.tile([_SEG, _H], f32)
            nc.vector.tensor_copy(o_t[:, 0:512], ps_a[:])
            nc.vector.tensor_copy(o_t[:, 512:768], ps_b[:])
            nc.sync.dma_start(out=out[:], in_=o_t[:])
    nc.compile()
    return nc


def _spans(entity_positions):
    ep = np.asarray(entity_positions).astype(np.int64)
    hs = np.clip(ep[:, 0], 0, _L - 1)
    he = np.maximum(hs, np.minimum(ep[:, 1], _L - 1))
    ts = np.clip(ep[:, 2], 0, _L - 1)
    te = np.maximum(ts, np.minimum(ep[:, 3], _L - 1))
    return hs, he, ts, te


def _plan(entity_positions):
    """Per-core batch assignment, window starts and weight rows."""
    hs, he, ts, te = _spans(entity_positions)

    runs = []
    usize = np.zeros(_B, np.int64)
    for b in range(_B):
        a0, a1, b0, b1 = hs[b], he[b], ts[b], te[b]
        if a0 > b0:
            a0, a1, b0, b1 = b0, b1, a0, a1
        if b0 <= a1 + 1:
            r = [(int(a0), int(max(a1, b1)))]
        else:
            r = [(int(a0), int(a1)), (int(b0), int(b1))]
        runs.append(r)
        usize[b] = sum(e - s + 1 for s, e in r)

    # greedy LPT assignment: heaviest batches first to the lightest core
    order = np.argsort(-usize, kind="stable")
    loads = np.zeros(_NCORES, np.int64)
    core_batches = [[] for _ in range(_NCORES)]
    for b in order:
        open_cores = [c for c in range(_NCORES) if len(core_batches[c]) < _BPC]
        c = min(open_cores, key=lambda i: loads[i])
        core_batches[c].append(int(b))
        loads[c] += usize[b]

    max_start = _BPC * _L - _K
    starts = [[] for _ in range(_NCORES)]   # window start rows
    wrows = [[] for _ in range(_NCORES)]    # per-window [K, SEG] weights
    for c in range(_NCORES):
        for lb, b in enumerate(core_batches[c]):
            base = lb * _L
            hw_ = np.float32(1.0 / (he[b] - hs[b] + 1))
            tw_ = np.float32(1.0 / (te[b] - ts[b] + 1))
            for (s, e) in runs[b]:
                t = s
                while t <= e:
                    wstart = min(base + t, max_start)
                    hi = min(e, wstart - base + _K - 1)
                    wr = np.zeros((_K, _SEG), np.float32)
                    r = wstart - base + np.arange(_K)
                    new = (r >= t) & (r <= hi)
                    wr[new & (r >= hs[b]) & (r <= he[b]), lb] = hw_
                    wr[new & (r >= ts[b]) & (r <= te[b]), _BPC + lb] = tw_
                    starts[c].append(wstart)
                    wrows[c].append(wr)
                    t = hi + 1

    n_win = max(len(s) for s in starts)
    n_inst = (n_win + 127) // 128
    n_slot = n_inst * 128  # layout slots; slots >= n_win are never gathered

    idx_mats, w_mats = [], []
    for c in range(_NCORES):
        pad = n_slot - len(starts[c])
        st = np.array(starts[c] + [0] * pad, np.int32)
        wr = np.stack(
            wrows[c] + [np.zeros((_K, _SEG), np.float32)] * pad
        )  # [n_slot, K, SEG]
        # window W_i -> instruction i = W_i // 128, partition p = W_i % 128
        idx_mat = np.ascontiguousarray(st.reshape(n_inst, 128).T)  # [128,n_inst]
        # w[p, ((i*K)+j)*SEG + m] = wr[i*128 + p, j, m]
        w_mat = np.ascontiguousarray(
            wr.reshape(n_inst, 128, _K, _SEG)
            .transpose(1, 0, 2, 3)
            .reshape(128, n_inst * _K * _SEG)
        )
        idx_mats.append(idx_mat)
        w_mats.append(w_mat)

    return core_batches, idx_mats, w_mats, n_win


def _run(sequence_output, entity_positions, trace=False, trace_cores=None):
    from concourse.bass_utils import run_bass_kernel_spmd

    x = np.ascontiguousarray(np.asarray(sequence_output), dtype=np.float32)
    core_batches, idx_mats, w_mats, n_win = _plan(entity_positions)

    if n_win not in _prog_cache:
        _prog_cache[n_win] = _build_program(n_win)
    nc = _prog_cache[n_win]

    in_maps = []
    for c in range(_NCORES):
        xc = np.ascontiguousarray(x[core_batches[c]]).reshape(_BPC * _L, _H)
        in_maps.append({"x": xc, "idx": idx_mats[c], "w": w_mats[c]})

    res = run_bass_kernel_spmd(
        nc, in_maps, list(range(_NCORES)), trace=trace,
        trace_cores=trace_cores,
    )

    head = np.zeros((_B, _H), np.float32)
    tail = np.zeros((_B, _H), np.float32)
    for c in range(_NCORES):
        o = res.results[c]["out"]
        for lb, b in enumerate(core_batches[c]):
            head[b] = o[lb]
            tail[b] = o[_BPC + lb]
    return (head, tail), res


def kernel(sequence_output, entity_positions):
    (head, tail), _ = _run(sequence_output, entity_positions)
    return head, tail


# revision 10
# speedup vs baseline: 1.5260x; 1.5260x over previous
"""Trainium2 Bass kernel for EntityMarker segment-reduce (span means).

Problem: sequence_output [128, 2048, 768] f32, entity_positions [128, 4] int.
For each batch b, compute the mean of sequence_output[b, s:e+1, :] for the
head span (cols 0,1) and tail span (cols 2,3), clamped like the reference.
Output: (head [128, 768], tail [128, 768]) f32.

Strategy (data-parallel over batch, 8 cores, load-balanced):
  - On host, compute clamped spans; per batch the union of the two spans is
    1-2 contiguous runs of rows. Only those rows (~26% of the tensor) are
    read on device. Batches are assigned to cores by greedy LPT on union
    size (16 batches/core) to balance per-core bytes.
  - Each run is covered by full K-row windows + leftover single rows.
    A gpsimd indirect DMA (InstDMACopy + dynamic offset) gathers one
    window per partition: out[p, :] = x[start[p] : start[p]+K] — K*3KB
    contiguous HBM reads per descriptor.
  - Interior windows lie fully inside a span, so their K rows share the
    weight 1/span_len: a DVE pairwise tree sums K rows -> 1 row per
    partition (fp32-exact, contiguous APs), then ONE PE matmul pair per
    gather accumulates weighted window-sums into PSUM [32, 768]
    (32 segments = 16 head + 16 tail per core). This keeps the fp32
    matmul (4 cycles/row) off the critical path.
  - Leftover rows (span_len % K per run) are gathered one row per
    partition and weighted per-row in the same PSUM accumulation.
  - The device program is UNIFORM across cores (SPMD); all data-dependence
    is carried via input tensors (x shard, window starts, weights).
"""

import os

import numpy as np

_B, _L, _H = 128, 2048, 768
_NCORES = 8
_BPC = _B // _NCORES  # batches per core
_SEG = 2 * _BPC       # segments per core: 16 head + 16 tail
_K = int(os.environ.get("KERNEL_K", "8"))   # rows per interior window
_GBUFS = int(os.environ.get("KERNEL_GBUFS", "3"))

_prog_cache = {}


def _build_program(n_wi, n_wr):
    import concourse.bass as bass
    import concourse.mybir as mybir
    from concourse import bacc, tile

    f32 = mybir.dt.float32
    i32 = mybir.dt.int32
    n_i = (n_wi + 127) // 128          # interior gather instructions
    p_i = n_wi - (n_i - 1) * 128       # partitions in last interior gather
    n_r = (n_wr + 127) // 128          # remainder gather instructions
    p_r = n_wr - (n_r - 1) * 128 if n_r else 0
    n_mm = n_i + n_r                   # matmul chunk slots

    nc = bacc.Bacc(None, target_bir_lowering=False)
    x = nc.declare_dram_parameter("x", [_BPC * _L, _H], f32, isOutput=False)
    idx = nc.declare_dram_parameter("idx", [128, n_mm], i32, isOutput=False)
    w = nc.declare_dram_parameter("w", [128, n_mm * _SEG], f32, isOutput=False)
    out = nc.declare_dram_parameter("out", [_SEG, _H], f32, isOutput=True)

    with tile.TileContext(nc) as tc:
        with (
            tc.tile_pool(name="const", bufs=1) as cpool,
            tc.tile_pool(name="gather", bufs=_GBUFS) as gpool,
            tc.tile_pool(name="tree", bufs=2) as tpool,
            tc.tile_pool(name="red", bufs=3) as rpool,
            tc.tile_pool(name="psum", bufs=1, space="PSUM") as ppool,
        ):
            idx_t = cpool.tile([128, n_mm], i32)
            nc.sync.dma_start(out=idx_t[:], in_=idx[:])
            w_t = cpool.tile([128, n_mm * _SEG], f32)
            nc.sync.dma_start(out=w_t[:], in_=w[:])

            ps_a = ppool.tile([_SEG, 512], f32)
            ps_b = ppool.tile([_SEG, 256], f32)

            def mm_pair(c, p, rhs):
                lhsT = w_t[:p, c * _SEG:(c + 1) * _SEG]
                nc.tensor.matmul(
                    ps_a[:], lhsT, rhs[:p, 0:512],
                    start=(c == 0), stop=(c == n_mm - 1))
                nc.tensor.matmul(
                    ps_b[:], lhsT, rhs[:p, 512:_H],
                    start=(c == 0), stop=(c == n_mm - 1))

            for t in range(n_i):
                p = 128 if t < n_i - 1 else p_i
                # NOTE: the gather out AP must be 2D — a 3D [128, K, H]
                # AP mis-gathers on HW (sim doesn't model it).
                g = gpool.tile([128, _K * _H], f32, tag="g")
                nc.gpsimd.indirect_dma_start(
                    out=g[:p],
                    out_offset=None,
                    in_=x[:],
                    in_offset=bass.IndirectOffsetOnAxis(
                        ap=idx_t[:p, t:t + 1], axis=0),
                )
                # pairwise tree: K rows -> 1 row, contiguous 768-blocks
                src = g
                k = _K
                while k > 2:
                    dst = tpool.tile([128, (k // 2) * _H], f32,
                                     tag=f"lvl{k}")
                    s3 = src[:p, 0:k * _H].rearrange(
                        "p (k2 two h) -> p k2 two h", two=2, h=_H)
                    nc.vector.tensor_add(
                        dst[:p].rearrange("p (k2 h) -> p k2 h", h=_H),
                        s3[:, :, 0, :], s3[:, :, 1, :])
                    src = dst
                    k //= 2
                red = rpool.tile([128, _H], f32, tag="red")
                nc.vector.tensor_add(
                    red[:p], src[:p, 0:_H], src[:p, _H:2 * _H])
                mm_pair(t, p, red)

            for t in range(n_r):
                p = 128 if t < n_r - 1 else p_r
                g1 = rpool.tile([128, _H], f32, tag="red")
                nc.gpsimd.indirect_dma_start(
                    out=g1[:p],
                    out_offset=None,
                    in_=x[:],
                    in_offset=bass.IndirectOffsetOnAxis(
                        ap=idx_t[:p, n_i + t:n_i + t + 1], axis=0),
                )
                mm_pair(n_i + t, p, g1)

            o_t = cpool.tile([_SEG, _H], f32)
            nc.vector.tensor_copy(o_t[:, 0:512], ps_a[:])
            nc.vector.tensor_copy(o_t[:, 512:_H], ps_b[:])
            nc.sync.dma_start(out=out[:], in_=o_t[:])
    nc.compile()
    return nc


def _spans(entity_positions):
    ep = np.asarray(entity_positions).astype(np.int64)
    hs = np.clip(ep[:, 0], 0, _L - 1)
    he = np.maximum(hs, np.minimum(ep[:, 1], _L - 1))
    ts = np.clip(ep[:, 2], 0, _L - 1)
    te = np.maximum(ts, np.minimum(ep[:, 3], _L - 1))
    return hs, he, ts, te


def _plan(entity_positions):
    """Per-core batch assignment, window starts and weights.

    Returns per-core interior windows (start row, segment, weight) and
    remainder rows (row, [(segment, weight)...]) in uniform-count layouts.
    """
    hs, he, ts, te = _spans(entity_positions)

    runs = []
    usize = np.zeros(_B, np.int64)
    for b in range(_B):
        a0, a1, b0, b1 = hs[b], he[b], ts[b], te[b]
        if a0 > b0:
            a0, a1, b0, b1 = b0, b1, a0, a1
        if b0 <= a1 + 1:
            r = [(int(a0), int(max(a1, b1)))]
        else:
            r = [(int(a0), int(a1)), (int(b0), int(b1))]
        runs.append(r)
        usize[b] = sum(e - s + 1 for s, e in r)

    # greedy LPT assignment: heaviest batches first to the lightest core
    order = np.argsort(-usize, kind="stable")
    loads = np.zeros(_NCORES, np.int64)
    core_batches = [[] for _ in range(_NCORES)]
    for b in order:
        open_cores = [c for c in range(_NCORES) if len(core_batches[c]) < _BPC]
        c = min(open_cores, key=lambda i: loads[i])
        core_batches[c].append(int(b))
        loads[c] += usize[b]

    # weight vector [SEG] for a row r of batch b at core-local slot lb
    def wvec(b, lb, r):
        v = np.zeros(_SEG, np.float32)
        if hs[b] <= r <= he[b]:
            v[lb] = np.float32(1.0 / (he[b] - hs[b] + 1))
        if ts[b] <= r <= te[b]:
            v[_BPC + lb] = np.float32(1.0 / (te[b] - ts[b] + 1))
        return v

    wins = [[] for _ in range(_NCORES)]   # (start_row, wrow[SEG])
    rems = [[] for _ in range(_NCORES)]   # (row, wrow[SEG])
    for c in range(_NCORES):
        for lb, b in enumerate(core_batches[c]):
            base = lb * _L
            for (s, e) in runs[b]:
                # split into subsegments of constant head/tail membership so
                # every full window has one weight vector for all its rows
                cuts = {s, e + 1}
                for v in (hs[b], he[b] + 1, ts[b], te[b] + 1):
                    if s < v <= e:
                        cuts.add(int(v))
                bounds = sorted(cuts)
                for ss, ee in zip(bounds[:-1], bounds[1:]):
                    ee -= 1  # inclusive
                    ln = ee - ss + 1
                    n_full = ln // _K
                    for i in range(n_full):
                        r0 = ss + i * _K
                        wins[c].append((base + r0, wvec(b, lb, r0)))
                    for r in range(ss + n_full * _K, ee + 1):
                        rems[c].append((base + r, wvec(b, lb, r)))

    n_wi = max(len(x) for x in wins)
    n_wr = max(len(x) for x in rems)
    n_i = (n_wi + 127) // 128
    n_r = (n_wr + 127) // 128
    n_mm = n_i + n_r

    idx_mats, w_mats = [], []
    for c in range(_NCORES):
        st = np.zeros(n_mm * 128, np.int32)
        wr = np.zeros((n_mm * 128, _SEG), np.float32)
        for i, (r0, wv) in enumerate(wins[c]):
            st[i] = r0
            wr[i] = wv
        for i, (r0, wv) in enumerate(rems[c]):
            st[n_i * 128 + i] = r0
            wr[n_i * 128 + i] = wv
        idx_mat = np.ascontiguousarray(st.reshape(n_mm, 128).T)
        w_mat = np.ascontiguousarray(
            wr.reshape(n_mm, 128, _SEG).transpose(1, 0, 2).reshape(128, -1))
        idx_mats.append(idx_mat)
        w_mats.append(w_mat)

    return core_batches, idx_mats, w_mats, n_wi, n_wr


def _run(sequence_output, entity_positions, trace=False, trace_cores=None):
    from concourse.bass_utils import run_bass_kernel_spmd

    x = np.ascontiguousarray(np.asarray(sequence_output), dtype=np.float32)
    core_batches, idx_mats, w_mats, n_wi, n_wr = _plan(entity_positions)

    key = (n_wi, n_wr)
    if key not in _prog_cache:
        _prog_cache[key] = _build_program(n_wi, n_wr)
    nc = _prog_cache[key]

    in_maps = []
    for c in range(_NCORES):
        xc = np.ascontiguousarray(x[core_batches[c]]).reshape(_BPC * _L, _H)
        in_maps.append({"x": xc, "idx": idx_mats[c], "w": w_mats[c]})

    res = run_bass_kernel_spmd(
        nc, in_maps, list(range(_NCORES)), trace=trace,
        trace_cores=trace_cores,
    )

    head = np.zeros((_B, _H), np.float32)
    tail = np.zeros((_B, _H), np.float32)
    for c in range(_NCORES):
        o = res.results[c]["out"]
        for lb, b in enumerate(core_batches[c]):
            head[b] = o[lb]
            tail[b] = o[_BPC + lb]
    return (head, tail), res


def kernel(sequence_output, entity_positions):
    (head, tail), _ = _run(sequence_output, entity_positions)
    return head, tail


# revision 11
# speedup vs baseline: 1.6707x; 1.0948x over previous
"""Trainium2 Bass kernel for EntityMarker segment-reduce (span means).

Problem: sequence_output [128, 2048, 768] f32, entity_positions [128, 4] int.
For each batch b, compute the mean of sequence_output[b, s:e+1, :] for the
head span (cols 0,1) and tail span (cols 2,3), clamped like the reference.
Output: (head [128, 768], tail [128, 768]) f32.

Strategy (data-parallel over batch, 8 cores, load-balanced):
  - On host, compute clamped spans; per batch the union of the two spans is
    1-2 contiguous runs of rows. Only those rows (~26% of the tensor) are
    read on device. Batches are assigned to cores by greedy LPT on union
    size (16 batches/core) to balance per-core bytes.
  - Each run is covered by full K-row windows + leftover single rows.
    A gpsimd indirect DMA (InstDMACopy + dynamic offset) gathers one
    window per partition: out[p, :] = x[start[p] : start[p]+K] — K*3KB
    contiguous HBM reads per descriptor.
  - Interior windows lie fully inside a span, so their K rows share the
    weight 1/span_len: a DVE pairwise tree sums K rows -> 1 row per
    partition (fp32-exact, contiguous APs), then ONE PE matmul pair per
    gather accumulates weighted window-sums into PSUM [32, 768]
    (32 segments = 16 head + 16 tail per core). This keeps the fp32
    matmul (4 cycles/row) off the critical path.
  - Leftover rows (span_len % K per run) are gathered one row per
    partition and weighted per-row in the same PSUM accumulation.
  - The device program is UNIFORM across cores (SPMD); all data-dependence
    is carried via input tensors (x shard, window starts, weights).
"""

import os

import numpy as np

_B, _L, _H = 128, 2048, 768
_NCORES = 8
_BPC = _B // _NCORES  # batches per core
_SEG = 2 * _BPC       # segments per core: 16 head + 16 tail
_K = int(os.environ.get("KERNEL_K", "8"))   # rows per interior window
_GBUFS = int(os.environ.get("KERNEL_GBUFS", "3"))

_prog_cache = {}


def _build_program(n_wi, n_wr):
    import concourse.bass as bass
    import concourse.mybir as mybir
    from concourse import bacc, tile

    f32 = mybir.dt.float32
    i32 = mybir.dt.int32
    n_i = (n_wi + 127) // 128          # interior gather instructions
    p_i = n_wi - (n_i - 1) * 128       # partitions in last interior gather
    n_r = (n_wr + 127) // 128          # remainder gather instructions
    p_r = n_wr - (n_r - 1) * 128 if n_r else 0
    n_mm = n_i + n_r                   # matmul chunk slots

    nc = bacc.Bacc(None, target_bir_lowering=False)
    x = nc.declare_dram_parameter("x", [_BPC * _L, _H], f32, isOutput=False)
    idx = nc.declare_dram_parameter("idx", [128, n_mm], i32, isOutput=False)
    w = nc.declare_dram_parameter("w", [128, n_mm * _SEG], f32, isOutput=False)
    out = nc.declare_dram_parameter("out", [_SEG, _H], f32, isOutput=True)

    with tile.TileContext(nc) as tc:
        with (
            tc.tile_pool(name="const", bufs=1) as cpool,
            tc.tile_pool(name="gather", bufs=_GBUFS) as gpool,
            tc.tile_pool(name="tree", bufs=2) as tpool,
            tc.tile_pool(name="red", bufs=3) as rpool,
            tc.tile_pool(name="psum", bufs=1, space="PSUM") as ppool,
        ):
            idx_t = cpool.tile([128, n_mm], i32)
            # load idx via the Pool engine's own SWDGE so the first gather's
            # descriptor generation isn't gated on a cross-engine HWDGE DMA
            nc.gpsimd.dma_start(out=idx_t[:], in_=idx[:])
            w_t = cpool.tile([128, n_mm * _SEG], f32)
            nc.sync.dma_start(out=w_t[:], in_=w[:])

            ps_a = ppool.tile([_SEG, 512], f32)
            ps_b = ppool.tile([_SEG, 256], f32)

            def mm_pair(c, p, rhs):
                lhsT = w_t[:p, c * _SEG:(c + 1) * _SEG]
                nc.tensor.matmul(
                    ps_a[:], lhsT, rhs[:p, 0:512],
                    start=(c == 0), stop=(c == n_mm - 1))
                nc.tensor.matmul(
                    ps_b[:], lhsT, rhs[:p, 512:_H],
                    start=(c == 0), stop=(c == n_mm - 1))

            for t in range(n_i):
                p = 128 if t < n_i - 1 else p_i
                # NOTE: the gather out AP must be 2D — a 3D [128, K, H]
                # AP mis-gathers on HW (sim doesn't model it).
                g = gpool.tile([128, _K * _H], f32, tag="g")
                nc.gpsimd.indirect_dma_start(
                    out=g[:p],
                    out_offset=None,
                    in_=x[:],
                    in_offset=bass.IndirectOffsetOnAxis(
                        ap=idx_t[:p, t:t + 1], axis=0),
                )
                # pairwise tree: K rows -> 1 row, contiguous 768-blocks
                src = g
                k = _K
                while k > 2:
                    dst = tpool.tile([128, (k // 2) * _H], f32,
                                     tag=f"lvl{k}")
                    s3 = src[:p, 0:k * _H].rearrange(
                        "p (k2 two h) -> p k2 two h", two=2, h=_H)
                    nc.vector.tensor_add(
                        dst[:p].rearrange("p (k2 h) -> p k2 h", h=_H),
                        s3[:, :, 0, :], s3[:, :, 1, :])
                    src = dst
                    k //= 2
                red = rpool.tile([128, _H], f32, tag="red")
                nc.vector.tensor_add(
                    red[:p], src[:p, 0:_H], src[:p, _H:2 * _H])
                mm_pair(t, p, red)

            for t in range(n_r):
                p = 128 if t < n_r - 1 else p_r
                g1 = rpool.tile([128, _H], f32, tag="red")
                nc.gpsimd.indirect_dma_start(
                    out=g1[:p],
                    out_offset=None,
                    in_=x[:],
                    in_offset=bass.IndirectOffsetOnAxis(
                        ap=idx_t[:p, n_i + t:n_i + t + 1], axis=0),
                )
                mm_pair(n_i + t, p, g1)

            o_t = cpool.tile([_SEG, _H], f32)
            nc.vector.tensor_copy(o_t[:, 0:512], ps_a[:])
            nc.vector.tensor_copy(o_t[:, 512:_H], ps_b[:])
            nc.sync.dma_start(out=out[:], in_=o_t[:])
    nc.compile()
    return nc


def _spans(entity_positions):
    ep = np.asarray(entity_positions).astype(np.int64)
    hs = np.clip(ep[:, 0], 0, _L - 1)
    he = np.maximum(hs, np.minimum(ep[:, 1], _L - 1))
    ts = np.clip(ep[:, 2], 0, _L - 1)
    te = np.maximum(ts, np.minimum(ep[:, 3], _L - 1))
    return hs, he, ts, te


def _plan(entity_positions):
    """Per-core batch assignment, window starts and weights.

    Returns per-core interior windows (start row, segment, weight) and
    remainder rows (row, [(segment, weight)...]) in uniform-count layouts.
    """
    hs, he, ts, te = _spans(entity_positions)

    runs = []
    usize = np.zeros(_B, np.int64)
    for b in range(_B):
        a0, a1, b0, b1 = hs[b], he[b], ts[b], te[b]
        if a0 > b0:
            a0, a1, b0, b1 = b0, b1, a0, a1
        if b0 <= a1 + 1:
            r = [(int(a0), int(max(a1, b1)))]
        else:
            r = [(int(a0), int(a1)), (int(b0), int(b1))]
        runs.append(r)
        usize[b] = sum(e - s + 1 for s, e in r)

    # greedy LPT assignment: heaviest batches first to the lightest core
    order = np.argsort(-usize, kind="stable")
    loads = np.zeros(_NCORES, np.int64)
    core_batches = [[] for _ in range(_NCORES)]
    for b in order:
        open_cores = [c for c in range(_NCORES) if len(core_batches[c]) < _BPC]
        c = min(open_cores, key=lambda i: loads[i])
        core_batches[c].append(int(b))
        loads[c] += usize[b]

    # weight vector [SEG] for a row r of batch b at core-local slot lb
    def wvec(b, lb, r):
        v = np.zeros(_SEG, np.float32)
        if hs[b] <= r <= he[b]:
            v[lb] = np.float32(1.0 / (he[b] - hs[b] + 1))
        if ts[b] <= r <= te[b]:
            v[_BPC + lb] = np.float32(1.0 / (te[b] - ts[b] + 1))
        return v

    wins = [[] for _ in range(_NCORES)]   # (start_row, wrow[SEG])
    rems = [[] for _ in range(_NCORES)]   # (row, wrow[SEG])
    for c in range(_NCORES):
        for lb, b in enumerate(core_batches[c]):
            base = lb * _L
            for (s, e) in runs[b]:
                # split into subsegments of constant head/tail membership so
                # every full window has one weight vector for all its rows
                cuts = {s, e + 1}
                for v in (hs[b], he[b] + 1, ts[b], te[b] + 1):
                    if s < v <= e:
                        cuts.add(int(v))
                bounds = sorted(cuts)
                for ss, ee in zip(bounds[:-1], bounds[1:]):
                    ee -= 1  # inclusive
                    ln = ee - ss + 1
                    n_full = ln // _K
                    for i in range(n_full):
                        r0 = ss + i * _K
                        wins[c].append((base + r0, wvec(b, lb, r0)))
                    for r in range(ss + n_full * _K, ee + 1):
                        rems[c].append((base + r, wvec(b, lb, r)))

    n_wi = max(len(x) for x in wins)
    n_wr = max(len(x) for x in rems)
    n_i = (n_wi + 127) // 128
    n_r = (n_wr + 127) // 128
    n_mm = n_i + n_r

    idx_mats, w_mats = [], []
    for c in range(_NCORES):
        st = np.zeros(n_mm * 128, np.int32)
        wr = np.zeros((n_mm * 128, _SEG), np.float32)
        for i, (r0, wv) in enumerate(wins[c]):
            st[i] = r0
            wr[i] = wv
        for i, (r0, wv) in enumerate(rems[c]):
            st[n_i * 128 + i] = r0
            wr[n_i * 128 + i] = wv
        idx_mat = np.ascontiguousarray(st.reshape(n_mm, 128).T)
        w_mat = np.ascontiguousarray(
            wr.reshape(n_mm, 128, _SEG).transpose(1, 0, 2).reshape(128, -1))
        idx_mats.append(idx_mat)
        w_mats.append(w_mat)

    return core_batches, idx_mats, w_mats, n_wi, n_wr


def _run(sequence_output, entity_positions, trace=False, trace_cores=None):
    from concourse.bass_utils import run_bass_kernel_spmd

    x = np.ascontiguousarray(np.asarray(sequence_output), dtype=np.float32)
    core_batches, idx_mats, w_mats, n_wi, n_wr = _plan(entity_positions)

    key = (n_wi, n_wr)
    if key not in _prog_cache:
        _prog_cache[key] = _build_program(n_wi, n_wr)
    nc = _prog_cache[key]

    in_maps = []
    for c in range(_NCORES):
        xc = np.ascontiguousarray(x[core_batches[c]]).reshape(_BPC * _L, _H)
        in_maps.append({"x": xc, "idx": idx_mats[c], "w": w_mats[c]})

    res = run_bass_kernel_spmd(
        nc, in_maps, list(range(_NCORES)), trace=trace,
        trace_cores=trace_cores,
    )

    head = np.zeros((_B, _H), np.float32)
    tail = np.zeros((_B, _H), np.float32)
    for c in range(_NCORES):
        o = res.results[c]["out"]
        for lb, b in enumerate(core_batches[c]):
            head[b] = o[lb]
            tail[b] = o[_BPC + lb]
    return (head, tail), res


def kernel(sequence_output, entity_positions):
    (head, tail), _ = _run(sequence_output, entity_positions)
    return head, tail
